# revision 30
# baseline (speedup 1.0000x reference)
"""CrossAttention kernel for 8 Trainium2 NeuronCores.

Data-parallel over batch: core b computes attention for tokens[b].
All device matmuls contract over the partition dim, so tokens are fed
pre-transposed ([hidden, T]) and scores/context vectors are kept in
transposed ([S, T] / [embed, T]) layout through the output projection,
which is also computed transposed ([hidden, T]); the host transposes
back and adds the output bias.

Softmax (over S=77) runs in the partition dim: exp on ScalarE (no
max-subtraction needed: scores ~ N(0,1) in f32), the denominator comes
from a ones-column appended to V (one extra PSUM row from the same
matmul), reciprocal on DVE (reciprocal_approx_fast: the denominators
are ~S*E[exp], far from the undefined edge cases), partition-broadcast
on GPSIMD, multiply on DVE. Scalar runs only Exp/Copy, which share one
activation-table set, so the table is loaded once.

Chunks of 512 tokens are software-pipelined: per chunk the emission
order is [heads(c), Qproj(c+1), outproj(c)] so the tensor engine fills
the normalize-tail window of chunk c with chunk c+1's Q projection.

Wq/Wk are zero-padded on the host from head_dim 80 to 96 so that each
head's K-slice of Q^T/K^T starts on a 32-aligned partition (PE array
row-group granularity).
"""

import numpy as np
import ml_dtypes

import concourse.bacc as bacc
import concourse.tile as tile
from concourse import mybir
import concourse.bass_utils as bass_utils

F32 = mybir.dt.float32
BF16 = mybir.dt.bfloat16

B, T, S = 8, 4096, 77
HID, EMB, CTX = 640, 640, 768
H, DH = 8, 80
DHP = 96            # head dim padded to a multiple of 32
EMBP = H * DHP      # 768 = 6 partition tiles of 128
KT_H = HID // 128   # 5  k-tiles for hidden-contraction
KT_C = CTX // 128   # 6  k-tiles for ctx-contraction
MT_Q = EMBP // 128  # 6  m-tiles of padded Q^T/K^T rows
NG = HID // 128     # 5  output column groups of 128
TCH = 512           # T chunk (one PSUM bank of f32)
NCH = T // TCH      # 8
P = 128
SCALE = 1.0 / np.sqrt(np.float32(DH))


def _part_cap(base):
    """Max partition count for an engine/PE access starting at `base`
    (within a 128-partition tile): base 0 -> 128, 64 -> 64, 32/96 -> 32."""
    b = base % P
    if b == 0:
        return P
    if b == 64:
        return 64
    assert b % 32 == 0, b
    return 32


def _matmul_segments(row0, nrows):
    """Split rows into (tile, a, b) pieces with legal partition base/count."""
    segs = []
    r = row0
    end = row0 + nrows
    while r < end:
        m, a = r // P, r % P
        c = min(end - r, _part_cap(a), P - a)
        segs.append((m, a, a + c))
        r += c
    return segs


def _write_segments(h):
    """Pieces of head h's padded rows in the stacked layout, split by
    the write-side quadrant caps. Yields (m, a, b, r0) with r0 the PSUM
    row the piece reads from. For odd heads the attn@V PSUM rows are
    rotated by 64 (matching the V-column rotation) so that every
    piece's PSUM read also starts at a legal quadrant base; the rotated
    pad rows read PSUM zeros, so the pad rows of ctx_v are zeroed. The
    broadcast operand is identical across partitions, so its read
    always starts at base 0."""
    segs = []
    off = 0
    while off < DHP:
        w = h * DHP + off
        m, a = w // P, w % P
        c = min(DHP - off, _part_cap(a), P - a)
        r0 = off if h % 2 == 0 else (off + 64) % DHP
        segs.append((m, a, a + c, r0))
        off += c
    return segs


def _build_program():
    nc = bacc.Bacc("TRN2", target_bir_lowering=False, debug=False, num_devices=B)

    tokT = nc.dram_tensor("tokT", [HID, T], BF16, kind="ExternalInput")
    ctxT = nc.dram_tensor("ctxT", [CTX, S], BF16, kind="ExternalInput")
    wqp = nc.dram_tensor("wqp", [HID, EMBP], BF16, kind="ExternalInput")
    wkp = nc.dram_tensor("wkp", [CTX, EMBP], BF16, kind="ExternalInput")
    wv = nc.dram_tensor("wv", [CTX, EMB], BF16, kind="ExternalInput")
    wo = nc.dram_tensor("wo", [EMBP, HID], BF16, kind="ExternalInput")
    outT = nc.dram_tensor("outT", [HID, T], BF16, kind="ExternalOutput")

    from contextlib import ExitStack
    with tile.TileContext(nc) as tc, ExitStack() as es:
        consts = es.enter_context(tc.tile_pool(name="consts", bufs=1))
        tok_pool = es.enter_context(tc.tile_pool(name="tok", bufs=3))
        qt_pool = es.enter_context(tc.tile_pool(name="qt", bufs=2))
        attn_pool = es.enter_context(tc.tile_pool(name="attn", bufs=4))
        r_pool = es.enter_context(tc.tile_pool(name="r", bufs=4))
        ctxv_pool = es.enter_context(tc.tile_pool(name="ctxv", bufs=2))
        out_pool = es.enter_context(tc.tile_pool(name="outp", bufs=2))
        ps_big = es.enter_context(tc.tile_pool(name="ps_big", bufs=3, space="PSUM"))
        ps_s = es.enter_context(tc.tile_pool(name="ps_s", bufs=2, space="PSUM"))
        ps_c = es.enter_context(tc.tile_pool(name="ps_c", bufs=3, space="PSUM"))

        # ---- load weights / context. Order matters: Qproj(0) needs only
        # wq+tok0, so those go first and the PE starts ~5us in; the
        # kt/v weights stream in behind them during the Q projection.
        wq_sb = consts.tile([P, KT_H, EMBP], BF16)
        wq_re = wqp.rearrange("(k p) n -> p k n", p=P)
        nc.sync.dma_start(out=wq_sb[:, :, 0:2 * P], in_=wq_re[:, :, 0:2 * P])

        tok_tiles = {}

        def emit_tok_dma(c):
            tok_sb = tok_pool.tile([P, KT_H, TCH], BF16, tag="tok", name=f"tok{c}")
            nc.sync.dma_start(
                out=tok_sb,
                in_=tokT.rearrange("(k p) t -> p k t", p=P)[
                    :, :, c * TCH:(c + 1) * TCH])
            tok_tiles[c] = tok_sb

        emit_tok_dma(0)
        nc.sync.dma_start(out=wq_sb[:, :, 2 * P:], in_=wq_re[:, :, 2 * P:])
        ctx_sb = consts.tile([P, KT_C, S], BF16)
        nc.sync.dma_start(out=ctx_sb, in_=ctxT.rearrange("(k p) s -> p k s", p=P))
        wk_sb = consts.tile([P, KT_C, EMBP], BF16)
        nc.sync.dma_start(out=wk_sb, in_=wkp.rearrange("(k p) n -> p k n", p=P))
        wv_sb = consts.tile([P, KT_C, EMB], BF16)
        nc.sync.dma_start(out=wv_sb, in_=wv.rearrange("(k p) n -> p k n", p=P))
        emit_tok_dma(1)
        wo_sb = consts.tile([P, MT_Q, HID], BF16)
        nc.sync.dma_start(out=wo_sb, in_=wo.rearrange("(k p) n -> p k n", p=P))


        def emit_qproj_mm(c, m):
            tok_sb = tok_tiles[c]
            ps = ps_big.tile([P, TCH], F32, tag="big", name=f"ps_q{c}_{m}")
            for k in range(KT_H):
                nc.tensor.matmul(
                    ps, wq_sb[:, k, m * P:(m + 1) * P], tok_sb[:, k, :],
                    start=(k == 0), stop=(k == KT_H - 1))
            return ps

        def emit_qproj(c, pre=(), qt_sb=None, mid=None):
            if qt_sb is None:
                qt_sb = qt_pool.tile([P, MT_Q, TCH], BF16, tag="qt",
                                     name=f"qt{c}")
            pre = dict(pre)
            for m in range(MT_Q):
                ps = pre.get(m)
                if ps is None:
                    ps = emit_qproj_mm(c, m)
                nc.scalar.copy(qt_sb[:, m, :], ps)
                if m == 1 and mid is not None:
                    # the trailing head's normalize is emitted here, after
                    # the first two qt copies, so those copies (and with
                    # them Qproj m2..m5 on the PE) are not queued behind
                    # the d-copy on the scalar engine
                    mid()
            return qt_sb

        emit_norm_fns = [None]

        def emit_heads(c, qt_sb, interleave=None):
            ctx_v = ctxv_pool.tile([P, MT_Q, TCH], BF16, tag="ctxv", name=f"ctxv{c}")

            def emit_norm(h, ps_cv):
                # r = 1/sum computed on ScalarE as exp(-ln(d)): Ln/Exp/Copy
                # share one activation table set, Ln reads the PSUM row
                # directly (no d-copy), and DVE keeps only the multiplies.
                # Broadcast across partitions on GPSIMD; the normalize
                # multiply reads the broadcast at partition 0 (all rows
                # identical), so only the writes are segmented.
                ln_sb = r_pool.tile([1, TCH], F32, tag="ln", name=f"ln{c}_{h}")
                nc.scalar.activation(ln_sb, ps_cv[DHP:DHP + 1, :],
                                     mybir.ActivationFunctionType.Ln)
                r_sb = r_pool.tile([1, TCH], F32, tag="r", name=f"r{c}_{h}")
                nc.scalar.activation(r_sb, ln_sb,
                                     mybir.ActivationFunctionType.Exp,
                                     scale=-1.0)
                rb_sb = r_pool.tile([DHP, TCH], F32, tag="rb", name=f"rb{c}_{h}")
                nc.gpsimd.partition_broadcast(rb_sb, r_sb)
                for (m, a, b, r0) in _write_segments(h):
                    nc.vector.tensor_mul(
                        ctx_v[a:b, m, :],
                        ps_cv[r0:r0 + (b - a), :],
                        rb_sb[0:(b - a), :])

            emit_norm_fns[0] = emit_norm
            pending = None
            for h in range(H):
                segs = _matmul_segments(h * DHP, DH)
                ps_sc = ps_s.tile([S, TCH], F32, tag="s", name=f"ps_s{c}_{h}")
                for i, (m, a, b) in enumerate(segs):
                    nc.tensor.matmul(
                        ps_sc, kt_sb[a:b, m, :], qt_sb[a:b, m, :],
                        start=(i == 0), stop=(i == len(segs) - 1),
                        tile_position=(a, 0))

                # attn^T = exp(scores / sqrt(DH))
                at_sb = attn_pool.tile([S, TCH], BF16, tag="at", name=f"at{c}_{h}")
                nc.scalar.activation(
                    at_sb, ps_sc, mybir.ActivationFunctionType.Exp,
                    scale=float(SCALE))

                # ctx_aug^T [DHP+1, TCH]; row 96 = sum(exp)
                ps_cv = ps_c.tile([DHP + 1, TCH], F32, tag="c", name=f"ps_c{c}_{h}")
                nc.tensor.matmul(ps_cv, v_sb[:, h, :], at_sb,
                                 start=True, stop=True)

                # the previous head's normalize is emitted AFTER this
                # head's exp so the scalar engine never makes the next
                # exp (and with it the attn@V matmul) wait on a d-copy
                if pending is not None:
                    emit_norm(*pending)
                pending = (h, ps_cv)
                if interleave is not None and h in (2, 4, 6):
                    interleave({2: 0, 4: 1, 6: 2}[h])
            return ctx_v, pending

        def emit_outproj(c, ctx_v):
            out_sb = out_pool.tile([P, NG, TCH], BF16, tag="out", name=f"out{c}")
            out_re = outT.rearrange("(g p) t -> p g t", p=P)
            for g in range(NG):
                ps_o = ps_big.tile([P, TCH], F32, tag="big", name=f"ps_o{c}_{g}")
                for k in range(MT_Q):
                    nc.tensor.matmul(
                        ps_o, wo_sb[:, k, g * P:(g + 1) * P], ctx_v[:, k, :],
                        start=(k == 0), stop=(k == MT_Q - 1))
                if g % 2 == 0:
                    nc.scalar.copy(out_sb[:, g, :], ps_o)
                else:
                    nc.vector.tensor_copy(out_sb[:, g, :], ps_o)
            nc.sync.dma_start(
                out=out_re[:, :, c * TCH:(c + 1) * TCH],
                in_=out_sb)

        qt_cur = emit_qproj(0)

        # ---- K^T [EMBP, S] as [128, 6, S] (padded-head rows) ----
        kt_sb = consts.tile([P, MT_Q, S], BF16)
        for m in range(MT_Q):
            ps = ps_big.tile([P, S], F32, tag="big", padded_shape=[None, TCH], name=f"ps_kt{m}")
            for k in range(KT_C):
                nc.tensor.matmul(
                    ps, wk_sb[:, k, m * P:(m + 1) * P], ctx_sb[:, k, :],
                    start=(k == 0), stop=(k == KT_C - 1))
            nc.vector.tensor_copy(kt_sb[:, m, :], ps)

        # ---- V [S, H, DHP+1]: col j holds V_h dim d with j = jmap_h(d),
        # col 96 = ones. The attn@V matmul then yields ctx dims on the
        # mapped rows, zeros elsewhere, and the softmax denominator in
        # (32-aligned) row 96. Odd heads rotate the dims by 64 so the
        # normalize pieces read PSUM from legal quadrant bases.
        v_sb = consts.tile([S, H, DHP + 1], BF16)
        nc.vector.memset(v_sb, 0.0)
        nc.vector.memset(v_sb[:, :, DHP:DHP + 1], 1.0)
        for h in range(H):
            ps = ps_c.tile([S, DH], F32, tag="c", padded_shape=[None, TCH], name=f"ps_v{h}")
            for k in range(KT_C):
                nc.tensor.matmul(
                    ps, ctx_sb[:, k, :], wv_sb[:, k, h * DH:(h + 1) * DH],
                    start=(k == 0), stop=(k == KT_C - 1))
            if h % 2 == 0:
                nc.vector.tensor_copy(v_sb[:, h, 0:DH], ps)
            else:
                nc.vector.tensor_copy(v_sb[:, h, 64:DHP], ps[:, 0:32])
                nc.vector.tensor_copy(v_sb[:, h, 0:DH - 32], ps[:, 32:DH])

        # ---- software-pipelined main loop. The first two m-tiles of
        # Qproj(c+1) are emitted (matmul only) inside the heads phase of
        # chunk c to fill the exp-wait bubbles on the tensor engine; their
        # PSUM->SBUF copies stay after the exps so the scalar engine's
        # exp cadence is not disturbed.
        for c in range(NCH):
            if c + 2 < NCH:
                emit_tok_dma(c + 2)
            pre = {}
            ctx_v, pending = emit_heads(
                c, qt_cur,
                interleave=(lambda m: pre.__setitem__(
                    m, emit_qproj_mm(c + 1, m)))
                if c + 1 < NCH else None)
            if c + 1 < NCH:
                qt_cur = emit_qproj(c + 1, pre=pre,
                                    mid=lambda: emit_norm_fns[0](*pending))
            else:
                emit_norm_fns[0](*pending)
            emit_outproj(c, ctx_v)

    nc.compile()
    return nc


_PROGRAM = None


def _get_program():
    global _PROGRAM
    if _PROGRAM is None:
        _PROGRAM = _build_program()
    return _PROGRAM


BF16_NP = ml_dtypes.bfloat16


def _pad_heads(w, dtype=np.float32):
    """[rows, H*DH] -> [rows, H*DHP] zero-padded per head."""
    rows = w.shape[0]
    wp = np.zeros((rows, EMBP), dtype)
    for h in range(H):
        wp[:, h * DHP:h * DHP + DH] = w[:, h * DH:(h + 1) * DH]
    return wp


def _pad_head_rows(w, dtype=np.float32):
    """[H*DH, cols] -> [H*DHP, cols] zero-padded per head."""
    wp = np.zeros((EMBP, w.shape[1]), dtype)
    for h in range(H):
        wp[h * DHP:h * DHP + DH] = w[h * DH:(h + 1) * DH]
    return wp


def build_in_maps(tokens, context, Wq, Wk, Wv, Wo):
    tokens = np.asarray(tokens, np.float32)
    context = np.asarray(context, np.float32)
    wqp = _pad_heads(np.asarray(Wq, np.float32)).astype(BF16_NP)
    wkp = _pad_heads(np.asarray(Wk, np.float32)).astype(BF16_NP)
    wv = np.ascontiguousarray(np.asarray(Wv, np.float32)).astype(BF16_NP)
    wo = _pad_head_rows(np.asarray(Wo, np.float32)).astype(BF16_NP)
    in_maps = []
    for b in range(B):
        in_maps.append({
            "tokT": np.ascontiguousarray(tokens[b].T).astype(BF16_NP),
            "ctxT": np.ascontiguousarray(context[b].T).astype(BF16_NP),
            "wqp": wqp, "wkp": wkp, "wv": wv, "wo": wo,
        })
    return in_maps


def postprocess(res, bo):
    """Gather per-core transposed bf16 outputs -> full [B, T, HID] f32."""
    bo = np.asarray(bo, np.float32)
    return np.stack([
        np.asarray(res.results[b]["outT"]).astype(np.float32).T + bo
        for b in range(B)])


def kernel(tokens, context, Wq, Wk, Wv, Wo, bo):
    nc = _get_program()
    in_maps = build_in_maps(tokens, context, Wq, Wk, Wv, Wo)
    res = bass_utils.run_bass_kernel_spmd(nc, in_maps, core_ids=list(range(B)))
    return postprocess(res, bo)
